# revision 9
# baseline (speedup 1.0000x reference)
"""ColBERT scoring kernel for Trainium2 (Bass/Tile), data-parallel over batch.

Reference computation (per batch b):
    Q = l2norm(q_hidden[b] @ W)                     # [LQ, DIM]
    D = l2norm((d_hidden[b] * mask[b,:,None]) @ W)  # [LD, DIM]
    score[b] = sum_q max_k (Q @ D.T)[q, k]

Sharding: batch dim B=64 split over 8 NeuronCores (8 batches/core), W replicated.

v2 design (vs the fp16 v1 at ~40us):
  - All HBM inputs in fp8 e4m3 (half the DMA bytes; tol is 2e-2, fp8 error
    lands ~3e-3).  W is pre-scaled by 16 on the host so its values sit in the
    e4m3 normal range; the scale cancels in the L2 normalization.
  - Projections contract over H=768 as 3 chunks of K=256 via DoubleRow fp8
    (2 weights/cell): operands are [128, 2, N] with the pair dim packing
    h = c*256 + j*128 + p.
  - Batches are fused column-wise: the 8 compacted doc batches form one
    2560-column stream processed in 512-col groups (PSUM bank = 512 fp32),
    so the d-projection is 15 matmuls instead of 48 and weight loads are
    amortized over 512-col moving operands.
  - Input DMAs split across the two HWDGE rings (scalar + sync engines) and
    issued first thing, so descriptor generation is parallel and transfers
    start immediately.
  - Dummy warm-up matmuls run during the initial DMA wait to flip the PE HAM
    clock gate (cold 1.2GHz -> warm 2.4GHz) before the real stream begins.
  - Doc-token norms: squares on GpSimd (SBUF-only engine, otherwise idle),
    partition-broadcast sum-of-squares via an all-ones stationary matmul,
    rsqrt on Scalar (one act table).  The normalization multiply is fused
    into the maxsim reduce: DVE tensor_tensor_reduce computes
    max_k(psim[:,k] * r[k]) in one pass, with the reduce init value 0
    reproducing the reference's zero (masked) columns in compact mode.
  - Query norms transpose-via-matmul ([dim,l] sumsq -> [l,b]) as in v1, and
    1/||q|| multiplies the per-q max after the reduce.
"""

import os

import numpy as np

B, LQ, LD, H, DIM = 64, 128, 512, 768, 128
NCORES = 8
BLOC = B // NCORES          # 8 batches per core
P = 128
CH = 3                      # contraction chunks of K=256 (DoubleRow pairs)
NQ = BLOC * LQ              # 1024 query columns per core
QG = NQ // 512              # query 512-col groups
EPS2 = 1e-24                # eps^2 so sqrt(ss + eps^2) ~ max(norm, 1e-12)
W_SCALE = 16.0              # host pre-scale so W lands in fp8 normal range

WARM_MM = int(os.environ.get("COLBERT_WARM", "5"))
USE_DR = os.environ.get("COLBERT_DR", "1") == "1"
USE_GPS = os.environ.get("COLBERT_GPS", "1") == "1"
USE_FUSED = os.environ.get("COLBERT_FUSED", "1") == "1"

_cache = {}


def _build(kd, compact):
    import concourse.bass as bass
    import concourse.tile as tile
    from concourse import bacc, mybir

    f32 = mybir.dt.float32
    f16 = mybir.dt.float16
    fp8 = mybir.dt.float8e4
    DR = mybir.MatmulPerfMode.DoubleRow

    ncol = BLOC * kd
    assert ncol % 512 == 0
    G = ncol // 512             # doc 512-col groups
    max_init = 0.0 if compact else -3.0e38

    nc = bacc.Bacc("TRN2", target_bir_lowering=False, debug=False,
                   num_devices=NCORES)

    qt = nc.dram_tensor("qt", [P, QG, CH, 2, 512], fp8, kind="ExternalInput").ap()
    dt = nc.dram_tensor("dt", [P, G, CH, 2, 512], fp8, kind="ExternalInput").ap()
    wt = nc.dram_tensor("wt", [P, CH, 2, DIM], fp8, kind="ExternalInput").ap()
    out = nc.dram_tensor("scores", [BLOC, 1], f32, kind="ExternalOutput").ap()

    with tile.TileContext(nc) as tc:
        with (
            tc.tile_pool(name="const", bufs=1) as const,
            tc.tile_pool(name="work", bufs=2) as work,
            tc.tile_pool(name="ps_proj", bufs=4, space="PSUM") as ps_proj,
            tc.tile_pool(name="ps_ssb", bufs=1, space="PSUM") as ps_ssb,
            tc.tile_pool(name="ps_sim", bufs=2, space="PSUM") as ps_sim,
        ):
            # ---- input DMAs first: split across the two HWDGE rings ----
            w_sb = const.tile([P, CH, 2, DIM], fp8)
            q_sb = const.tile([P, QG, CH, 2, 512], fp8)
            d_sb = const.tile([P, G, CH, 2, 512], fp8)
            nc.sync.dma_start(out=w_sb, in_=wt)
            nc.scalar.dma_start(out=q_sb[:, 0], in_=qt[:, 0])
            nc.sync.dma_start(out=q_sb[:, 1], in_=qt[:, 1])
            for g in range(G):
                eng = nc.scalar if g % 2 == 0 else nc.sync
                eng.dma_start(out=d_sb[:, g], in_=dt[:, g])

            # ---- constants ----
            junk = const.tile([P, 2, 640], fp8)
            nc.vector.memset(junk, 0.0)
            ones_pk = const.tile([P, P], f16)
            nc.gpsimd.memset(ones_pk, 1.0)
            ones_c1 = const.tile([P, 1], f16)
            nc.gpsimd.memset(ones_c1, 1.0)
            ones_f32 = const.tile([P, 1], f32)
            nc.gpsimd.memset(ones_f32, 1.0)
            eps_c = const.tile([P, 1], f32)
            nc.gpsimd.memset(eps_c, EPS2)

            qT_all = const.tile([P, NQ], f16)       # unnormalized Q^T cast
            sqq = const.tile([P, NQ], f16)          # squares of qT
            pc = const.tile([P, ncol], f16)         # unnormalized P_d^T cast
            r_all = const.tile([P, ncol], f32)      # 1/||d_k||, bcast over p
            rq = const.tile([P, BLOC], f32)         # 1/||q_l|| as [l, b]
            m_cols = const.tile([P, BLOC], f32)     # per-q maxsim
            scores_cols = const.tile([P, BLOC], f32)

            # ---- PE warm-up: junk matmuls flip the HAM clock gate while
            # the first input DMAs are in flight ----
            for i in range(WARM_MM):
                pw = ps_ssb.tile([P, 512], f32, name=f"pw{i}", tag="ssb")
                if USE_DR:
                    nc.tensor.matmul(pw, junk[:, :, :DIM], junk[:, :, DIM:],
                                     start=True, stop=True, perf_mode=DR)
                else:
                    nc.tensor.matmul(pw, junk[:, 0, :DIM], junk[:, 0, DIM:],
                                     start=True, stop=True)

            # ---- projections: lhsT.T @ rhs, contracting H as K=256
            # DoubleRow chunks (or 6 plain fp8 K=128 chunks) ----
            def proj(dst, src_g):
                if USE_DR:
                    for c in range(CH):
                        nc.tensor.matmul(
                            dst, w_sb[:, c], src_g[:, c],
                            start=(c == 0), stop=(c == CH - 1), perf_mode=DR,
                        )
                else:
                    for c in range(CH):
                        for j in range(2):
                            nc.tensor.matmul(
                                dst, w_sb[:, c, j], src_g[:, c, j],
                                start=(c == 0 and j == 0),
                                stop=(c == CH - 1 and j == 1),
                            )

            pq = []
            for g in range(QG):
                pqg = ps_proj.tile([P, 512], f32, name=f"pq{g}", tag="proj")
                proj(pqg, q_sb[:, g])
                pq.append(pqg)

            pd = []

            def d_group(g):
                pdg = ps_proj.tile([P, 512], f32, name=f"pd{g}", tag="proj")
                proj(pdg, d_sb[:, g])
                pd.append(pdg)
                gsl = slice(g * 512, (g + 1) * 512)
                # PSUM -> SBUF cast (also the sim moving operand)
                nc.vector.tensor_copy(pc[:, gsl], pdg)
                # squares on gpsimd (SBUF only), fp16
                sqg = work.tile([P, 512], f16, name=f"sq{g}", tag="sq")
                eng = nc.gpsimd if USE_GPS else nc.vector
                eng.tensor_mul(sqg, pc[:, gsl], pc[:, gsl])
                return sqg

            def d_sumsq(g, sqg):
                gsl = slice(g * 512, (g + 1) * 512)
                pssb = ps_ssb.tile([P, 512], f32, name=f"pssb{g}", tag="ssb")
                nc.tensor.matmul(pssb, ones_pk, sqg, start=True, stop=True)
                nc.scalar.activation(
                    r_all[:, gsl], pssb,
                    mybir.ActivationFunctionType.Abs_reciprocal_sqrt,
                    bias=eps_c)

            def q_post(g):
                gsl = slice(g * 512, (g + 1) * 512)
                nc.vector.tensor_copy(qT_all[:, gsl], pq[g])
                eng = nc.gpsimd if USE_GPS else nc.vector
                eng.tensor_mul(sqq[:, gsl], qT_all[:, gsl], qT_all[:, gsl])

            def q_norms():
                pssq = ps_ssb.tile([P, BLOC], f32, tag="misc")
                for b in range(BLOC):
                    nc.tensor.matmul(
                        pssq[:, b:b + 1],
                        sqq[:, b * LQ:(b + 1) * LQ],
                        ones_c1, start=True, stop=True,
                    )
                nc.scalar.activation(
                    rq, pssq,
                    mybir.ActivationFunctionType.Abs_reciprocal_sqrt,
                    bias=eps_c)

            def sim(b):
                bsl = slice(b * kd, (b + 1) * kd)
                psim = ps_sim.tile([P, 512], f32, name=f"psim{b}", tag="psim")
                nc.tensor.matmul(
                    psim[:, :kd],
                    qT_all[:, b * LQ:(b + 1) * LQ],
                    pc[:, bsl], start=True, stop=True,
                )
                scr = work.tile([P, 512], f16, name=f"scr{b}", tag="scr")
                # m[l] = max_k(sim[l,k] * r[k]); init 0 models the masked
                # (zeroed) doc columns the compact packing dropped
                if USE_FUSED:
                    nc.vector.tensor_tensor_reduce(
                        out=scr[:, :kd], in0=psim[:, :kd], in1=r_all[:, bsl],
                        scale=1.0, scalar=max_init,
                        op0=mybir.AluOpType.mult, op1=mybir.AluOpType.max,
                        accum_out=m_cols[:, b:b + 1])
                else:
                    nc.vector.tensor_mul(scr[:, :kd], psim[:, :kd],
                                         r_all[:, bsl])
                    nc.vector.tensor_reduce(
                        m_cols[:, b:b + 1], scr[:, :kd],
                        mybir.AxisListType.X, mybir.AluOpType.max)
                    if compact:
                        nc.vector.tensor_scalar_max(m_cols[:, b:b + 1],
                                                    m_cols[:, b:b + 1], 0.0)
                nc.vector.tensor_mul(scores_cols[:, b:b + 1],
                                     m_cols[:, b:b + 1], rq[:, b:b + 1])

            # ---- schedule: d-projections interleaved with post work so the
            # PE queue never stalls on a not-yet-ready dependency ----
            q_post(0)
            q_post(1)
            sq0 = d_group(0)
            sq1 = d_group(1)
            sq2 = d_group(2)
            q_norms()
            d_sumsq(0, sq0)
            sqs = {0: sq0, 1: sq1, 2: sq2}
            nb = 0  # batches with sim issued
            for g in range(3, G):
                sqs[g] = d_group(g)
                d_sumsq(g - 2, sqs[g - 2])
                # sim for batches fully covered by casts/norms so far
                while (nb + 1) * kd <= (g - 1) * 512 and nb < BLOC:
                    sim(nb)
                    nb += 1
            d_sumsq(G - 2, sqs[G - 2])
            d_sumsq(G - 1, sqs[G - 1])
            while nb < BLOC:
                sim(nb)
                nb += 1

            # ---- final: per-batch sum over LQ (partition reduce by matmul)
            pfin = ps_ssb.tile([BLOC, 1], f32, tag="misc")
            nc.tensor.matmul(pfin, scores_cols, ones_f32, start=True, stop=True)
            scores_sb = work.tile([BLOC, 1], f32, tag="fin", bufs=1)
            nc.vector.tensor_copy(scores_sb, pfin)
            nc.sync.dma_start(out=out, in_=scores_sb)

    nc.compile()
    return nc


def _pack_cols(arr2d, g, p=P):
    """[ncols, H] fp32 -> [128, ncols/512, 3, 2, 512] with h = c*256+j*128+p."""
    n = arr2d.shape[0]
    return arr2d.reshape(g, 512, CH, 2, p).transpose(4, 0, 2, 3, 1)


def kernel(q_hidden, d_hidden, W, doc_mask):
    import ml_dtypes
    from concourse.bass_utils import run_bass_kernel_spmd

    q_hidden = np.asarray(q_hidden, dtype=np.float32)
    d_hidden = np.asarray(d_hidden, dtype=np.float32)
    W = np.asarray(W, dtype=np.float32)
    doc_mask = np.asarray(doc_mask)

    fp8 = ml_dtypes.float8_e4m3

    # compaction: keep only unmasked doc tokens; zero-padded to kd. Reference
    # zero columns re-enter the score through the max-reduce's 0 init.
    K_CAP = 320
    counts = (doc_mask != 0).sum(axis=1)
    compact = counts.max() <= K_CAP and bool(np.isin(doc_mask, (0, 1)).all())
    kd = K_CAP if compact else LD
    key = ("nc", kd, compact)
    if key not in _cache:
        _cache[key] = _build(kd=kd, compact=compact)
    nc = _cache[key]

    if compact:
        d_m = np.zeros((B, kd, H), dtype=np.float32)
        mask_b = doc_mask != 0
        for b in range(B):
            sel = d_hidden[b][mask_b[b]]
            d_m[b, :len(sel)] = sel
    else:
        d_m = d_hidden * doc_mask[..., None].astype(np.float32)

    G = BLOC * kd // 512
    wt = np.ascontiguousarray(
        (W * W_SCALE).reshape(CH, 2, P, DIM).transpose(2, 0, 1, 3)
    ).astype(fp8)
    in_maps = []
    for c in range(NCORES):
        sl = slice(c * BLOC, (c + 1) * BLOC)
        qtc = np.ascontiguousarray(
            _pack_cols(q_hidden[sl].reshape(BLOC * LQ, H), QG)).astype(fp8)
        dtc = np.ascontiguousarray(
            _pack_cols(d_m[sl].reshape(BLOC * kd, H), G)).astype(fp8)
        in_maps.append({"qt": qtc, "dt": dtc, "wt": wt})

    trace = os.environ.get("COLBERT_TRACE", "0") == "1"
    res = run_bass_kernel_spmd(nc, in_maps, core_ids=list(range(NCORES)),
                               trace=trace)
    _cache["last_results"] = res
    return np.concatenate([r["scores"].reshape(BLOC) for r in res.results])
